# revision 18
# baseline (speedup 1.0000x reference)
"""Trainium2 Bass kernel for nn_NodeAttention (hypergraph message passing).

Math (reference):
    w      = sigmoid(x @ attn_w.T + attn_b)[:, 0]          # per-edge weight (M == N)
    e_feat = Binv * segsum_by_edge(x[node_idx])            # node -> hyperedge
    D      = segsum_by_node(w[edge_idx]);  Dinv = 1/D (0 where D==0)
    out    = (Dinv * segsum_by_node(e_feat[edge_idx])) @ lin_w.T + bias

Distribution (replicated gather + local segment sum, 8 cores):
core c owns edge rows [c*6250, (c+1)*6250) for the node->edge phase and the
same node range for the edge->node phase.

Both phases are pure sequential device streams: the host performs the
per-entry replicated gather (phase A: rows of x; phase B: rows of the
device-computed ea table) into partition-major [P, T, C] bf16 tile images of
128-entry tiles grouped by destination window, plus a [P, T] image of
relative destination columns (-1 for pads). Destinations are bin-packed into
200 windows of <=32 dests / <=512 entries per core (uniform 4 tiles per
window, ~2% padding; falls back to contiguous windows if packing fails);
the host unpermutes the outputs. The device streams tiles, builds one-hot
destination columns in batches of 32 tiles with one custom DVE op
(body=eq(Src0, Src1), paged [P, S, 32] iota vs per-page dst scalar —
~35 ns/tile), and segment-sums via PE matmuls accumulated in PSUM
supertiles [C, 256] (8 windows per PSUM bank, feature-major), so lin_w is
one stationary-weight matmul per supertile and PSUM drains are 25 wide ACT
ops per phase. The phase-B lin matmul is deferred one supertile so its
psum->SBUF copy hides under the next supertile's segment matmuls. Binv and
Dinv (host bincounts; D uses device-computed attention scores) are folded
into the phase-B stream scaling on the host.

Precision: streams/one-hots/matmul operands bf16, accumulation fp32 PSUM,
outputs written bf16 (ea and out slabs), final output assembled fp32.
"""

import os
import sys
import heapq
from contextlib import ExitStack

import numpy as np
import ml_dtypes

for _p in (
    "/root/.axon_site",
    "/root/.axon_site/_ro/trn_rl_repo",
    "/root/.axon_site/_ro/pypackages",
):
    if os.path.isdir(_p) and _p not in sys.path:
        sys.path.append(_p)

import concourse.bass as bass
import concourse.mybir as mybir
import concourse.tile as tile
from concourse import bacc
from concourse.bass_utils import run_bass_kernel_spmd

P = 128
N_NODES = 50000
N_EDGES = 50000
C = 128
NCORES = 8
SLAB = N_NODES // NCORES           # 6250 rows owned per core
W = 32                             # destinations per window
SG = 8                             # windows per PSUM supertile (256 dests)
NBINS = 200                        # packed windows per core
BIN_CAP = 512                      # max entries per packed window (4 tiles)
SMAX = 32                          # one-hot tiles per custom DVE op
CHUNK_T = 64                       # stream tiles per DMA chunk

F32 = mybir.dt.float32
BF16 = mybir.dt.bfloat16
BF = ml_dtypes.bfloat16

TRACE = False
LAST_EXEC_NS = {}

_PROGRAMS = {}
_ONEHOT_OP = None


def _onehot_op():
    """Runtime-register the batched one-hot custom DVE op:
    out[p, s, n] = (in0[p, s, n] == in1[p, s, 0]). uops sha is computed at
    registration so compile()'s drift check is self-consistent."""
    global _ONEHOT_OP
    if _ONEHOT_OP is not None:
        return _ONEHOT_OP
    from concourse.dve_spec import Spec, Src0, Src1, eq, lower
    from concourse.dve_ops import (
        DveOp, DveOpSpec, OPS, _SUB_OPCODE_FOR_NAME, _CUSTOM_DVE_ROW_BASE,
    )

    name = "ONE_HOT_EQ_ANT"
    if name in _SUB_OPCODE_FOR_NAME:
        _ONEHOT_OP = next(o for o in OPS if o.name == name)
        return _ONEHOT_OP
    spec = Spec(
        body=eq(Src0, Src1),
        reference=lambda in0, in1, s0, s1, imm2: (
            in0.astype(np.float32)
            == np.broadcast_to(in1, in0.shape).astype(np.float32)
        ).astype(np.float32),
    )
    row = _CUSTOM_DVE_ROW_BASE + len(OPS)
    assert row < 0x20, "custom DVE opcode rows exhausted"
    shas = {}
    for ver in ("v3", "v4"):
        uops = lower(spec, ver=ver)
        shas[ver] = DveOpSpec(name=name, opcode=row, uops=uops, rd1_en=True).sha(ver)
    op = DveOp(name, spec, subdim=False, uops_sha=shas)
    OPS.append(op)
    _SUB_OPCODE_FOR_NAME[name] = row
    _ONEHOT_OP = op
    return op


# ----------------------------------------------------------------------------
# Host-side planning
# ----------------------------------------------------------------------------

def _pack_positions(dst_ids, nbins):
    """Per core: bin-pack local dests (by descending degree) into nbins
    windows of <=W dests and <=BIN_CAP entries. Returns pos [NCORES, SLAB]
    (dest -> window*W + slot) or None if packing fails for any core."""
    dst_ids = np.asarray(dst_ids, np.int64)
    if nbins * W < SLAB:
        return None
    pos = np.empty((NCORES, SLAB), np.int64)
    for c in range(NCORES):
        local = dst_ids[(dst_ids >= c * SLAB) & (dst_ids < (c + 1) * SLAB)] - c * SLAB
        deg = np.bincount(local, minlength=SLAB)
        order = np.argsort(-deg, kind="stable")
        bins_e = np.zeros(nbins, np.int64)
        bins_d = np.zeros(nbins, np.int64)
        heap = [(0, i) for i in range(nbins)]
        heapq.heapify(heap)
        for dd in order:
            g = int(deg[dd])
            tmp = []
            placed = False
            while heap:
                e, i = heapq.heappop(heap)
                if bins_d[i] < W and e + g <= BIN_CAP:
                    pos[c, dd] = i * W + bins_d[i]
                    bins_e[i] = e + g
                    bins_d[i] += 1
                    if bins_d[i] < W:
                        heapq.heappush(heap, (e + g, i))
                    placed = True
                    break
                tmp.append((e, i))
            for t in tmp:
                heapq.heappush(heap, t)
            if not placed:
                return None
    return pos


def _plan(dst_ids):
    """Entries grouped by (dest core, window), padded to shared per-window
    128-entry tile counts (max across cores). Tries bin-packed layouts, then
    the contiguous fallback. Returns entry permutation, per-entry
    (core, lane, tile) placement, tile counts, the [NC, P, T] relative-dest
    image (-1 for pads), and the global dest -> output-column map."""
    dst_ids = np.asarray(dst_ids, np.int64)
    core = dst_ids // SLAB
    local = dst_ids - core * SLAB
    pos = _pack_positions(dst_ids, NBINS)
    wpc = NBINS
    if pos is None:
        pos = _pack_positions(dst_ids, NBINS + 4)
        wpc = NBINS + 4
    if pos is None:
        wpc = (SLAB + W - 1) // W
        p_ent = local
        colmap = np.arange(NCORES)[:, None] * (wpc * W) + np.arange(SLAB)[None, :]
    else:
        p_ent = pos[core, local]
        colmap = np.arange(NCORES)[:, None] * (wpc * W) + pos
    w = p_ent // W
    rel = (p_ent - w * W).astype(np.float32)
    key = core * wpc + w
    order = np.argsort(key, kind="stable")
    k = key[order]
    counts = np.bincount(k, minlength=NCORES * wpc).reshape(NCORES, wpc)
    t_w = np.maximum(1, ((counts.max(axis=0) + P - 1) // P)).astype(np.int64)
    t_off = np.concatenate([[0], np.cumsum(t_w)])
    T = int(t_off[-1])
    flat = counts.reshape(-1)
    starts = np.cumsum(flat) - flat
    rank = np.arange(k.shape[0], dtype=np.int64) - starts[k]
    cc = k // wpc
    ww = k - cc * wpc
    tl = t_off[ww] + rank // P
    lane = rank - (rank // P) * P
    dst_img = np.full((NCORES, P, T), -1.0, np.float32)
    dst_img[cc, lane, tl] = rel[order]
    return (order, cc, lane, tl, tuple(int(t) for t in t_w), T,
            dst_img.astype(BF), colmap.reshape(-1))


def _stream_image(cc, lane, tl, T, rows_bf):
    """Scatter sorted per-entry feature rows into the padded partition-major
    [NC, P, T, C] bf16 stream image."""
    img = np.zeros((NCORES, P, T, C), BF)
    img[cc, lane, tl] = rows_bf
    return img


def _supertiles(t_w):
    """[(w0, w1, t0, tiles), ...] groups of up to SG windows per PSUM bank."""
    wpc = len(t_w)
    t_off = [0]
    for t in t_w:
        t_off.append(t_off[-1] + t)
    out = []
    for g0 in range(0, wpc, SG):
        g1 = min(g0 + SG, wpc)
        out.append((g0, g1, t_off[g0], t_off[g1] - t_off[g0]))
    return out


def _chunks(sts):
    """Group whole supertiles into DMA chunks of <= CHUNK_T tiles; the first
    chunks are small so compute starts as soon as possible."""
    total = sum(s[3] for s in sts)
    out = []
    i = 0
    n = len(sts)
    done = 0
    while i < n:
        cap = CHUNK_T
        if not out:
            cap = max(sts[0][3], 1)     # single supertile
        elif len(out) == 1:
            cap = max(2 * sts[i][3], 1)
        elif total - done <= 2 * CHUNK_T:
            cap = max(CHUNK_T // 2, sts[i][3])
        j = i
        tiles = 0
        while j < n and tiles + sts[j][3] <= cap:
            tiles += sts[j][3]
            j += 1
        if j == i:
            j = i + 1
            tiles = sts[i][3]
        out.append((i, j, tiles))
        done += tiles
        i = j
    return out


# ----------------------------------------------------------------------------
# Bass program (shared template for both phases)
# ----------------------------------------------------------------------------

def _new_nc():
    return bacc.Bacc(
        "TRN2",
        target_bir_lowering=False,
        debug=False,
        enable_asserts=False,
        num_devices=NCORES,
    )


def _phase_program(t_w, mode):
    """mode 'A': stream x rows grouped by edge; emit ea slab [C, wpc*W] bf16
    (raw segment sums, packed positions) and scores wslab [1, SLAB] f32.
    mode 'B': stream host-scaled ea rows grouped by node; apply lin_w and
    bias; emit outslab [C, wpc*W] bf16 (output transposed, packed)."""
    onehot = _onehot_op()
    t_w = tuple(int(t) for t in t_w)
    wpc = len(t_w)
    slabp = wpc * W
    T = sum(t_w)
    t_off = [0]
    for t in t_w:
        t_off.append(t_off[-1] + t)
    sts = _supertiles(t_w)
    chunks = _chunks(sts)

    nc = _new_nc()
    xg = nc.dram_tensor("xg", [P, T * C], BF16, kind="ExternalInput").ap()
    dst = nc.dram_tensor("dst", [P, T], BF16, kind="ExternalInput").ap()
    if mode == "A":
        xsl = nc.dram_tensor("xsl", [C, SLAB], BF16, kind="ExternalInput").ap()
        acol = nc.dram_tensor("acol", [C, 1], BF16, kind="ExternalInput").ap()
        bcol = nc.dram_tensor("bcol", [1, 1], F32, kind="ExternalInput").ap()
        easlab = nc.dram_tensor(
            "easlab", [C, slabp], BF16, kind="ExternalOutput"
        ).ap()
        wout = nc.dram_tensor("wout", [1, SLAB], F32, kind="ExternalOutput").ap()
    else:
        wt = nc.dram_tensor("wt", [C, C], BF16, kind="ExternalInput").ap()
        biasc = nc.dram_tensor("biasc", [C, 1], F32, kind="ExternalInput").ap()
        outslab = nc.dram_tensor(
            "outslab", [C, slabp], BF16, kind="ExternalOutput"
        ).ap()

    with tile.TileContext(nc) as tc:
        with ExitStack() as ctx:
            const = ctx.enter_context(tc.tile_pool(name="const", bufs=1))
            spool = ctx.enter_context(tc.tile_pool(name="stream", bufs=6))
            opool = ctx.enter_context(tc.tile_pool(name="oh", bufs=6))
            wpool = ctx.enter_context(tc.tile_pool(name="work", bufs=4))
            ps1 = ctx.enter_context(tc.tile_pool(name="ps1", bufs=4, space="PSUM"))
            ps2 = ctx.enter_context(tc.tile_pool(name="ps2", bufs=2, space="PSUM"))

            iota_i = const.tile([P, SMAX * W], mybir.dt.int32)
            nc.gpsimd.iota(
                iota_i[:].rearrange("p (s n) -> p s n", n=W),
                pattern=[[0, SMAX], [1, W]], base=0, channel_multiplier=0,
            )
            iota_rep = const.tile([P, SMAX * W], BF16)
            nc.vector.tensor_copy(iota_rep[:], iota_i[:])

            dst_sb = const.tile([P, T], BF16)
            nc.scalar.dma_start(out=dst_sb[:], in_=dst[:])

            if mode == "A":
                xsl_sb = const.tile([C, SLAB], BF16)
                nc.scalar.dma_start(out=xsl_sb[:], in_=xsl[:])
                acol_sb = const.tile([C, 1], BF16)
                nc.scalar.dma_start(out=acol_sb[:], in_=acol[:])
                bcol_sb = const.tile([1, 1], F32)
                nc.scalar.dma_start(out=bcol_sb[:], in_=bcol[:])
                w_sb = const.tile([1, SLAB], F32)
            else:
                wt_sb = const.tile([C, C], BF16)
                nc.scalar.dma_start(out=wt_sb[:], in_=wt[:])
                bias_sb = const.tile([C, 1], F32)
                nc.scalar.dma_start(out=bias_sb[:], in_=biasc[:])

            SCW = 512
            n_score = (SLAB + SCW - 1) // SCW
            score_k = 0

            def emit_score(k):
                rows = min(SCW, SLAB - k * SCW)
                pss = ps2.tile([1, SCW], F32)
                nc.tensor.matmul(
                    out=pss[0:1, :rows],
                    lhsT=acol_sb[:],
                    rhs=xsl_sb[:, k * SCW : k * SCW + rows],
                    start=True,
                    stop=True,
                )
                nc.scalar.activation(
                    w_sb[0:1, k * SCW : k * SCW + rows],
                    pss[0:1, :rows],
                    mybir.ActivationFunctionType.Sigmoid,
                    bias=bcol_sb[0:1, 0:1],
                    scale=1.0,
                )

            pending = None   # (supertile index, sb1 tile, width) awaiting lin
            obuf_st = [None, 0]   # rolling 2-supertile output buffer (mode B)
            ebuf_st = [None, 0]   # rolling 2-supertile ea buffer (mode A)

            def emit_lin(st_i, sb1, width):
                po = ps2.tile([C, SG * W], F32)
                nc.tensor.matmul(
                    out=po[:, :width], lhsT=wt_sb[:], rhs=sb1[:, :width],
                    start=True, stop=True,
                )
                half = st_i % 2
                if half == 0 or obuf_st[0] is None:
                    obuf_st[0] = wpool.tile([C, 2 * SG * W], BF16, tag="obuf", name="obuf")
                    obuf_st[1] = st_i
                obuf = obuf_st[0]
                off = half * SG * W
                nc.scalar.activation(
                    obuf[:, off : off + width], po[:, :width],
                    mybir.ActivationFunctionType.Identity,
                    bias=bias_sb[:, 0:1], scale=1.0,
                )
                if half == 1 or st_i == len(sts) - 1:
                    base = sts[obuf_st[1]][0] * W
                    span = (st_i % 2) * SG * W + width
                    nc.gpsimd.dma_start(
                        out=outslab[:, base : base + span],
                        in_=obuf[:, :span],
                    )
                    obuf_st[0] = None

            for ci0, ci1, ctiles in chunks:
                c0 = sts[ci0][2]
                xga = spool.tile([P, CHUNK_T * C], BF16, tag="xga")
                nc.sync.dma_start(
                    out=xga[:, : ctiles * C],
                    in_=xg[:, c0 * C : (c0 + ctiles) * C],
                )
                # batched one-hot groups covering this chunk's tile range
                groups = {}
                g0 = 0
                while g0 < ctiles:
                    gs = min(SMAX, ctiles - g0)
                    s_g = opool.tile([P, SMAX * W], BF16, tag="s")
                    nc.vector._custom_dve(
                        onehot,
                        out=s_g[:, : gs * W].rearrange("p (s n) -> p s n", n=W),
                        in0=iota_rep[:, : gs * W].rearrange(
                            "p (s n) -> p s n", n=W
                        ),
                        in1=dst_sb[:, c0 + g0 : c0 + g0 + gs].to_broadcast(
                            [P, gs, W]
                        ),
                    )
                    groups[g0 // SMAX] = s_g
                    g0 += gs
                for st_i in range(ci0, ci1):
                    w0, w1, st_t0, st_tiles = sts[st_i]
                    width = (w1 - w0) * W
                    ps = ps1.tile([C, SG * W], F32)
                    for w in range(w0, w1):
                        n_t = t_w[w]
                        cw = (w - w0) * W
                        for j in range(n_t):
                            lt = t_off[w] + j - c0
                            s_g = groups[lt // SMAX]
                            col = lt - (lt // SMAX) * SMAX
                            nc.tensor.matmul(
                                out=ps[:, cw : cw + W],
                                lhsT=xga[:, lt * C : (lt + 1) * C],
                                rhs=s_g[:, col * W : (col + 1) * W],
                                start=(j == 0),
                                stop=(j == n_t - 1),
                            )
                    if mode == "A":
                        half = st_i % 2
                        if half == 0 or ebuf_st[0] is None:
                            ebuf_st[0] = wpool.tile(
                                [C, 2 * SG * W], BF16, tag="ebuf", name="ebuf"
                            )
                            ebuf_st[1] = st_i
                        ebuf = ebuf_st[0]
                        off = half * SG * W
                        nc.scalar.copy(ebuf[:, off : off + width], ps[:, :width])
                        if half == 1 or st_i == len(sts) - 1:
                            base = sts[ebuf_st[1]][0] * W
                            span = (st_i % 2) * SG * W + width
                            nc.gpsimd.dma_start(
                                out=easlab[:, base : base + span],
                                in_=ebuf[:, :span],
                            )
                            ebuf_st[0] = None
                        while score_k < n_score and score_k * 2 * SG <= w0:
                            emit_score(score_k)
                            score_k += 1
                    else:
                        if pending is not None:
                            emit_lin(*pending)
                        sb1 = wpool.tile([C, SG * W], BF16, tag="sb1")
                        nc.scalar.copy(sb1[:, :width], ps[:, :width])
                        pending = (st_i, sb1, width)

            if mode == "A":
                while score_k < n_score:
                    emit_score(score_k)
                    score_k += 1
                nc.gpsimd.dma_start(out=wout[:], in_=w_sb[:])
            else:
                if pending is not None:
                    emit_lin(*pending)
    nc.compile()
    return nc


def _program(mode, t_w):
    key = (mode, t_w)
    if key not in _PROGRAMS:
        _PROGRAMS[key] = _phase_program(t_w, mode)
    return _PROGRAMS[key]


# ----------------------------------------------------------------------------
# Entry point
# ----------------------------------------------------------------------------

def _run(nc, in_maps, label):
    kwargs = {}
    if TRACE:
        kwargs = dict(trace=True, trace_cores=[0])
    res = run_bass_kernel_spmd(nc, in_maps, core_ids=list(range(NCORES)), **kwargs)
    if res.exec_time_ns is not None:
        LAST_EXEC_NS[label] = res.exec_time_ns
    return res.results


def kernel(x, hyperedge_index, attn_w, attn_b, lin_w, bias):
    x = np.ascontiguousarray(np.asarray(x, dtype=np.float32))
    he = np.asarray(hyperedge_index)
    node_idx = he[0].astype(np.int64)
    edge_idx = he[1].astype(np.int64)
    attn_w = np.asarray(attn_w, dtype=np.float32)
    attn_b = np.asarray(attn_b, dtype=np.float32)
    lin_w = np.asarray(lin_w, dtype=np.float32)
    bias = np.asarray(bias, dtype=np.float32)

    x_bf = x.astype(BF)

    # --- host planning ------------------------------------------------------
    ordA, ccA, laneA, tlA, t_wA, TA, dstA, colA = _plan(edge_idx)
    ordB, ccB, laneB, tlB, t_wB, TB, dstB, colB = _plan(node_idx)

    xgA = _stream_image(ccA, laneA, tlA, TA, x_bf[node_idx[ordA]])

    bdeg = np.bincount(edge_idx, minlength=N_EDGES)
    binv = np.where(bdeg > 0, 1.0 / np.maximum(bdeg, 1), 0.0).astype(np.float32)

    # x.T slabs for the on-device attention scores
    xslT = np.ascontiguousarray(
        x_bf.reshape(NCORES, SLAB, C).transpose(0, 2, 1)
    )  # [NC, C, SLAB]
    a_col = np.ascontiguousarray(attn_w.reshape(C, 1)).astype(BF)
    b_col = np.full((1, 1), float(attn_b.reshape(-1)[0]), np.float32)

    # --- phase A: node -> edge (raw segment sums + scores) ------------------
    nc_a = _program("A", t_wA)
    in_maps_a = [
        {
            "xg": xgA[c].reshape(P, TA * C),
            "dst": dstA[c],
            "xsl": xslT[c],
            "acol": a_col,
            "bcol": b_col,
        }
        for c in range(NCORES)
    ]
    res_a = _run(nc_a, in_maps_a, "A")

    ea_cols = np.concatenate([r["easlab"] for r in res_a], axis=1)
    ea_rows = np.ascontiguousarray(ea_cols[:, colA].T)              # [N, C] bf16
    w_full = np.concatenate([r["wout"][0] for r in res_a])          # [N] f32

    D = np.bincount(node_idx, weights=w_full[edge_idx].astype(np.float64),
                    minlength=N_NODES)
    dinv = np.where(D > 0, 1.0 / np.maximum(D, 1e-300), 0.0).astype(np.float32)

    srcB = edge_idx[ordB]
    scale = binv[srcB] * dinv[node_idx[ordB]]
    rowsB = (ea_rows[srcB].astype(np.float32) * scale[:, None]).astype(BF)
    xgB = _stream_image(ccB, laneB, tlB, TB, rowsB)

    wt_host = np.ascontiguousarray(lin_w.T).astype(BF)      # [C_in, C_out]
    bias_col = np.ascontiguousarray(bias.reshape(C, 1)).astype(np.float32)

    # --- phase B: edge -> node (scaled segment sums, lin_w, bias) -----------
    nc_b = _program("B", t_wB)
    in_maps_b = [
        {
            "xg": xgB[c].reshape(P, TB * C),
            "dst": dstB[c],
            "wt": wt_host,
            "biasc": bias_col,
        }
        for c in range(NCORES)
    ]
    res_b = _run(nc_b, in_maps_b, "B")
    out_cols = np.concatenate([r["outslab"] for r in res_b], axis=1)
    return np.ascontiguousarray(out_cols[:, colB].T.astype(np.float32))
